# revision 42
# baseline (speedup 1.0000x reference)
"""AdaConv2D (instance-norm -> grouped 3x3 conv -> grouped 1x1 conv -> bias) on 8 TRN2 cores.

v2 strategy (norm FOLDED into the conv; pure data parallel, 1 sample/core):

  Host-side prep (inside kernel(), before launch):
  - Fuse the grouped 1x1 conv into the grouped 3x3 conv:
    eff[g,j,i,kh,kw] = sum_m pw[g,j,m]*dw[g,m,i,kh,kw].
  - Pack eff into block-diagonal 64x64 bf16 lhsT tiles (16 groups of 4x4 per
    tile), one per (chunk, tap); x converted to bf16.
  - NEW: pack 3 static "constant" matrices per chunk (A_K = sum of all 9 taps,
    A_L = sum of kw=0 taps, A_R = sum of kw=2 taps) as [128,128] block-diag
    lhsT with duplicated column halves.

  Device: instance-norm is NEVER applied to x.  Instead:
  - out = Conv(eff*rstd_i, y) - K + edge fixups, where y = raw x with the
    top/bottom halo ROWS filled with the per-channel MEAN (so vertical taps
    read exactly mean -> zero contribution after the K subtraction), and the
    kw=+-1 taps keep the shrunk-AP trick at the W edges.
  - Per chunk at runtime: rstd scales the 9 tap tiles (one gpsimd
    tensor_scalar over [128,576]); 3 tiny matmuls compute
    {K, C_L, C_R}[j] = A_c x (mean*rstd) with R-halves selected by a masked
    rhs and duplicated across partition halves by the lhsT packing; K folds
    into the evac bias; C_L/C_R are added to output cols 0/127 by 4 small
    gpsimd fixup ops per drain piece.
  - This removes the whole normalize pass (~60us of gpsimd work that gated
    conv start and chunk boundaries in v1) and shrinks the prologue: chunk-0
    stats run per-DMA-slice on DVE(bn_stats)+ACT(accum) in parallel.
  - Chain uses Sqrt (ACT) + reciprocal (DVE) only - avoids the Ln/Exp
    activation-table thrash (1.3us/chunk each reload).
  - Engine roles: PE conv (64x64 quadrant tiles, 4 concurrent streams, the
    ~216ns/window issue sweet spot; 32x32 is issue-bound at ~20ns/instr);
    ACT stats (Copy/Square accum); DVE psum evac only (+chain tails);
    GpSimd chain/wscale/halo/fixups; Sync DMA.
  - Last chunk drains in quarters then eighths to shrink the tail.
"""
import os
import sys
import numpy as np
import ml_dtypes

if "/opt/trn_rl_repo" not in sys.path:
    sys.path.insert(0, "/opt/trn_rl_repo")

B, C, H, W = 8, 512, 128, 128
HW = H * W            # 16384
NCH = 4               # 128-channel chunks per sample
NTAP = 9
ROWS_PAD = H + 2      # 130 rows of 128 in padded SBUF layout
PADF = ROWS_PAD * W   # 16640 elems per partition
EPS = 1e-7
# taps ordered so the first three are dw=0 (full-width writes -> correct PSUM init)
TAPS = [(0, 1), (1, 1), (2, 1), (0, 0), (1, 0), (2, 0), (0, 2), (1, 2), (2, 2)]

_CACHE = {}


def _build_program():
    import concourse.bass as bass
    import concourse.tile as tile
    from concourse import bacc, mybir

    f32 = mybir.dt.float32
    bf16 = mybir.dt.bfloat16
    MULT = mybir.AluOpType.mult
    ADD = mybir.AluOpType.add
    nc = bacc.Bacc("TRN2", target_bir_lowering=False, debug=False,
                   enable_asserts=False, num_devices=8)

    x_d = nc.dram_tensor("x", [C, HW], bf16, kind="ExternalInput")
    w_d = nc.dram_tensor("w", [128, NCH * NTAP * 64], bf16, kind="ExternalInput")
    w2_d = nc.dram_tensor("w2", [128, NCH * 3 * 128], bf16, kind="ExternalInput")
    b_d = nc.dram_tensor("bias", [128, 10], f32, kind="ExternalInput")
    out_d = nc.dram_tensor("out", [C, HW], bf16, kind="ExternalOutput")

    # store view: [cc, Ch(spatial half), hh(drain half), p, R, e(4096)]
    out_v = out_d[:].rearrange("(a R p) (Ch hh e) -> a Ch hh p R e", a=NCH, R=2,
                               p=64, Ch=2, hh=2, e=4096)

    with tile.TileContext(nc) as tc:
        with (
            tc.tile_pool(name="xpool", bufs=3) as xpool,
            tc.tile_pool(name="wpool", bufs=1) as wpool,
            tc.tile_pool(name="wppool", bufs=2) as wppool,
            tc.tile_pool(name="spool", bufs=3) as spool,
            tc.tile_pool(name="opool", bufs=2) as opool,
            tc.tile_pool(name="psum", bufs=7, space=bass.MemorySpace.PSUM) as pspool,
            tc.tile_pool(name="kcpsum", bufs=1, space=bass.MemorySpace.PSUM) as kcpool,
        ):
            w_sb = wpool.tile([128, NCH * NTAP * 64], bf16)
            w2_sb = wpool.tile([128, NCH * 3 * 128], bf16)
            bias_sb = wpool.tile([128, 10], f32)
            trash0 = wpool.tile([128, 4096], bf16)
            trash1 = wpool.tile([128, 4096], bf16)
            trash = [trash0, trash1]
            ones_row = wpool.tile([128, W], f32)
            dum_a = wpool.tile([128, 1], bf16)
            dum_x = wpool.tile([128, 512], bf16)
            magic_sb = wpool.tile([128, 1], mybir.dt.int32)
            dum_b = wpool.tile([128, 1], f32)
            dum_c = wpool.tile([128, 1], f32)

            st = {}  # per-chunk small tiles

            def create_xt(cc):
                xt = xpool.tile([128, PADF], bf16, tag="xt", name=f"xt{cc}")
                st[cc] = {"xt": xt}

            def issue_load(cc, nsl=4):
                xt = st[cc]["xt"]
                sz = HW // nsl
                for k in range(nsl):
                    nc.sync.dma_start(xt[:, W + k * sz: W + (k + 1) * sz],
                                      x_d[cc * 128:(cc + 1) * 128,
                                          k * sz:(k + 1) * sz])

            def emit_load(cc, nsl=4):
                create_xt(cc)
                issue_load(cc, nsl)

            def emit_stats_bn(cc, slot):
                # steady stats: DVE bn_stats, 4 blocks of 512 per slot (8 slots)
                s = st[cc]
                if "st6" not in s:
                    s["st6"] = spool.tile([128, 32 * 6], f32, tag="st6",
                                          name=f"s6{cc}")
                xt = s["xt"]
                for j in range(4):
                    blk = slot * 4 + j
                    nc.vector.bn_stats(s["st6"][:, blk * 6:(blk + 1) * 6],
                                       xt[:, W + blk * 512: W + (blk + 1) * 512])

            def emit_chain_tail(cc, s, mean_ap, var_ap):
                # rstd = rsqrt(var*N/(N-1)) via bit-trick seed + 2 Newton
                # iterations, ALL on DVE: no ACT op anywhere in the chain, so
                # the in-order ACT evac stream can never block on statistics.
                # (seed err 3.4% -> 1.8e-3 -> 4.7e-6 relative; budget is 2e-2)
                v = nc.vector
                i32 = mybir.dt.int32
                vc = spool.tile([128, 1], f32, tag="vc", name=f"vc{cc}")
                v.tensor_scalar_mul(vc[:], var_ap, float(HW) / float(HW - 1))
                sh = spool.tile([128, 1], i32, tag="sh", name=f"sh{cc}")
                v.tensor_scalar(sh[:], vc[:].bitcast(i32), 1, None,
                                op0=mybir.AluOpType.logical_shift_right)
                r0 = spool.tile([128, 1], i32, tag="r0", name=f"r0{cc}")
                v.tensor_sub(r0[:], magic_sb[:], sh[:])
                r = r0[:].bitcast(f32)
                for it in range(2):
                    a = spool.tile([128, 1], f32, tag=f"nwa{it}",
                                   name=f"na{cc}_{it}")
                    v.tensor_mul(a[:], r, r)
                    u = spool.tile([128, 1], f32, tag=f"nwu{it}",
                                   name=f"nu{cc}_{it}")
                    v.scalar_tensor_tensor(u[:], a[:], -0.5, vc[:],
                                           op0=MULT, op1=MULT)
                    w = spool.tile([128, 1], f32, tag=f"nww{it}",
                                   name=f"nw{cc}_{it}")
                    v.tensor_scalar_add(w[:], u[:], 1.5)
                    rn = spool.tile([128, 1], f32, tag=f"nwr{it}",
                                    name=f"nr{cc}_{it}")
                    v.tensor_mul(rn[:], r, w[:])
                    r = rn[:]
                mrp = spool.tile([128, 1], f32, tag="mrp", name=f"mp{cc}")
                v.tensor_mul(mrp[:], mean_ap, r)
                s["mean_ap"] = mean_ap
                s["rstd_ap"] = r[:, 0:1]
                s["mrp_ap"] = mrp[:, 0:1]

            def emit_chain_bn(cc):
                # steady chain: bn_aggr (DVE) -> sqrt (ACT) -> DVE tail
                s = st[cc]
                mv = spool.tile([128, 2], f32, tag="mv", name=f"mv{cc}")
                nc.vector.bn_aggr(mv[:], s["st6"][:].rearrange(
                    "p (h s) -> p h s", s=6))
                emit_chain_tail(cc, s, mv[:, 0:1], mv[:, 1:2])

            def emit_chain_mix(cc, alpha, npairs, nblk):
                # merge DVE bn_stats (fraction `alpha`) with ACT accum pairs
                s = st[cc]
                acc = s["acc"]
                v = nc.vector
                mv = spool.tile([128, 2], f32, tag="mv", name=f"mv{cc}")
                v.bn_aggr(mv[:], s["st6"][:, 0:nblk * 6].rearrange(
                    "p (h s) -> p h s", s=6))
                if npairs == 3:
                    t4 = spool.tile([128, 2], f32, tag="t4", name=f"t4{cc}")
                    sm = spool.tile([128, 2], f32, tag="sm", name=f"sm{cc}")
                    v.tensor_add(t4[:, 0:2], acc[:, 0:2], acc[:, 2:4])
                    v.tensor_add(sm[:, 0:2], t4[:, 0:2], acc[:, 4:6])
                    sm_ap = sm
                else:
                    sm_ap = acc
                mb = spool.tile([128, 1], f32, tag="mb", name=f"mb{cc}")
                v.tensor_scalar_mul(mb[:], sm_ap[:, 0:1], 1.0 / HW)
                mean = spool.tile([128, 1], f32, tag="mean", name=f"me{cc}")
                v.scalar_tensor_tensor(mean[:], mv[:, 0:1], alpha, mb[:],
                                       op0=MULT, op1=ADD)
                m2a = spool.tile([128, 1], f32, tag="m2a", name=f"ma{cc}")
                v.tensor_mul(m2a[:], mv[:, 0:1], mv[:, 0:1])
                e2a = spool.tile([128, 1], f32, tag="e2a", name=f"ea{cc}")
                v.tensor_scalar(e2a[:], mv[:, 1:2], m2a[:, 0:1], alpha,
                                op0=ADD, op1=MULT)
                e2b = spool.tile([128, 1], f32, tag="e2b", name=f"eb{cc}")
                v.tensor_scalar_mul(e2b[:], sm_ap[:, 1:2], 1.0 / HW)
                ex2 = spool.tile([128, 1], f32, tag="ex2", name=f"ex{cc}")
                v.tensor_add(ex2[:], e2a[:], e2b[:])
                m2 = spool.tile([128, 1], f32, tag="m2", name=f"m2{cc}")
                v.tensor_mul(m2[:], mean[:], mean[:])
                var = spool.tile([128, 1], f32, tag="var", name=f"va{cc}")
                v.tensor_sub(var[:], ex2[:], m2[:])
                emit_chain_tail(cc, s, mean[:, 0:1], var[:, 0:1])

            def emit_stats_pair(cc, lo, size):
                # one ACT Copy+Square accum pair over xt[lo:lo+size]
                s = st[cc]
                if "acc" not in s:
                    s["acc"] = spool.tile([128, 2], f32, tag="acc2",
                                          name=f"a2{cc}")
                sl = s["xt"][:, W + lo: W + lo + size]
                nc.scalar.activation(trash[0][:, 0:size], sl,
                                     mybir.ActivationFunctionType.Copy,
                                     accum_out=s["acc"][:, 0:1])
                nc.scalar.activation(trash[1][:, 0:size], sl,
                                     mybir.ActivationFunctionType.Square,
                                     accum_out=s["acc"][:, 1:2])

            def emit_post_chain(cc):
                # DVE: scaled weights + masked mr vector; ACT: mean halo rows
                # (Copy with per-partition scale).  gpsimd is too slow here
                # (8.4us for the [128,576] scale vs 0.36us on DVE).
                s = st[cc]
                wp = wppool.tile([128, NTAP * 64], bf16, tag="wp",
                                 name=f"wp{cc}")
                nc.vector.tensor_scalar_mul(wp[:],
                                            w_sb[:, cc * 576:(cc + 1) * 576],
                                            s["rstd_ap"])
                xt = s["xt"]
                nc.vector.tensor_scalar_mul(xt[:, 0:W], ones_row[:],
                                            s["mean_ap"])
                nc.vector.tensor_scalar_mul(xt[:, PADF - W:PADF], ones_row[:],
                                            s["mean_ap"])
                # mrr[:, R] = mask_R * mrp (masks are host-packed bias cols 8/9)
                mrr = spool.tile([128, 2], bf16, tag="mrr", name=f"mr{cc}")
                for R in range(2):
                    nc.vector.tensor_scalar_mul(mrr[:, R:R + 1],
                                                bias_sb[:, 8 + R:9 + R],
                                                s["mrp_ap"])
                s["wp"] = wp
                s["mrr"] = mrr

            def emit_consts(cc):
                # 3 tiny matmuls: kc[:, 2c:2c+2] = A_c^T x mrr  (c: K, C_L, C_R)
                s = st[cc]
                kcp = kcpool.tile([128, 6], f32, tag="kc", name=f"kp{cc}")
                for c in range(3):
                    nc.tensor.matmul(
                        kcp[:, 2 * c:2 * c + 2],
                        w2_sb[:, (cc * 3 + c) * 128:(cc * 3 + c + 1) * 128],
                        s["mrr"][:, 0:2], start=True, stop=True)
                kc = spool.tile([128, 6], f32, tag="kcs", name=f"kc{cc}")
                nc.vector.tensor_scalar_add(kc[:], kcp[:], 0.0)
                bk = spool.tile([128, 2], f32, tag="bk", name=f"bk{cc}")
                for R in range(2):
                    nc.vector.tensor_sub(bk[:, R:R + 1],
                                         bias_sb[:, cc * 2 + R:cc * 2 + R + 1],
                                         kc[:, R:R + 1])
                s["kc"] = kc
                s["bk"] = bk

            def emit_span_mms(cc, q):
                # span q: four 64x64 array tiles = 2 channel sub-chunk PAIRS
                # (row groups R, 16 groups block-diag each) x 2 spatial halves
                # (col groups C); C covers spatial tile 16C + q
                s = st[cc]
                xt = s["xt"]
                wp = s["wp"]
                pb = [pspool.tile([128, 512], f32, tag="pb",
                                  name=f"pb{cc}_{q}_{R}") for R in range(2)]
                for ti, (dh, dwi) in enumerate(TAPS):
                    start, stop = (ti == 0), (ti == NTAP - 1)
                    tapi = dh * 3 + dwi
                    for R in range(2):
                        lhsT = wp[64 * R:64 * R + 64,
                                  tapi * 64:tapi * 64 + 64]
                        for Cg in range(2):
                            t = 16 * Cg + q
                            base = (4 * t + dh) * W
                            outp = pb[R][64 * Cg:64 * Cg + 64, :]
                            tp = (64 * R, 64 * Cg)
                            if dwi == 1:
                                nc.tensor.matmul(
                                    outp, lhsT,
                                    xt[64 * R:64 * R + 64, base:base + 512],
                                    start=start, stop=stop, tile_position=tp)
                            else:
                                o3 = outp.rearrange("p (h w) -> p h w", w=W)
                                r3 = xt[64 * R:64 * R + 64,
                                        base:base + 512].rearrange(
                                            "p (h w) -> p h w", w=W)
                                if dwi == 0:   # dw=-1
                                    nc.tensor.matmul(
                                        o3[:, :, 1:W], lhsT, r3[:, :, 0:W - 1],
                                        start=start, stop=stop,
                                        skip_group_check=True, tile_position=tp)
                                else:          # dw=+1
                                    nc.tensor.matmul(
                                        o3[:, :, 0:W - 1], lhsT, r3[:, :, 1:W],
                                        start=start, stop=stop,
                                        skip_group_check=True, tile_position=tp)
                return pb

            def emit_evac(cc, q, pb, om):
                # psum drain on ACT (Identity: out = psum + bias); DVE is the
                # stats engine now, ACT the evac engine
                bk = st[cc]["bk"]
                for R in range(2):
                    dst = om[:, R * 8192 + q * 512: R * 8192 + q * 512 + 512]
                    nc.scalar.activation(dst, pb[R][:, :],
                                         mybir.ActivationFunctionType.Identity,
                                         bias=bk[:, R:R + 1])

            def emit_fixups(cc, om, qlo, qhi):
                # add C_L / C_R to output cols 0 / 127 of spans [qlo, qhi)
                # on ACT: in-order right after the evacs that wrote the band
                kc = st[cc]["kc"]
                IDENT = mybir.ActivationFunctionType.Identity
                for R in range(2):
                    band = om[:, R * 8192 + qlo * 512: R * 8192 + qhi * 512]
                    b3 = band.rearrange("p (x w) -> p x w", w=W)
                    nc.scalar.activation(b3[:, :, 0:1], b3[:, :, 0:1], IDENT,
                                         bias=kc[:, 2 + R:3 + R])
                    nc.scalar.activation(b3[:, :, W - 1:W], b3[:, :, W - 1:W],
                                         IDENT, bias=kc[:, 4 + R:5 + R])

            def emit_out(cc, om, hh):
                # drains issue from the (otherwise idle) gpsimd queue so their
                # fixup-dependency gates never block the load stream on sync
                for Cg in range(2):
                    nc.gpsimd.dma_start(
                        out_v[cc, Cg, hh],
                        om[64 * Cg:64 * Cg + 64, :].rearrange(
                            "p (R hh e) -> p R hh e", hh=2, e=4096)[:, :, hh, :])

            # finer store views for the last chunk's drains (shrinks the tail)
            out_v4 = out_d[:].rearrange("(a R p) (Ch qq e) -> a Ch qq p R e",
                                        a=NCH, R=2, p=64, Ch=2, qq=4, e=2048)
            out_v8 = out_d[:].rearrange("(a R p) (Ch qq e) -> a Ch qq p R e",
                                        a=NCH, R=2, p=64, Ch=2, qq=8, e=1024)

            def emit_out4(cc, om, part):
                for Cg in range(2):
                    nc.gpsimd.dma_start(
                        out_v4[cc, Cg, part],
                        om[64 * Cg:64 * Cg + 64, :].rearrange(
                            "p (R qq e) -> p R qq e", qq=4, e=2048)[:, :, part, :])

            def emit_out8(cc, om, part):
                for Cg in range(2):
                    nc.gpsimd.dma_start(
                        out_v8[cc, Cg, part],
                        om[64 * Cg:64 * Cg + 64, :].rearrange(
                            "p (R qq e) -> p R qq e", qq=8, e=1024)[:, :, part, :])

            T0, TC = 28.0, 32.0   # rough sim-time anchors (us)

            def wu(us):
                return tc.tile_wait_until(us / 1000.0)

            # ---- prologue ----
            nc.gpsimd.memset(ones_row[:], 1.0)
            nc.gpsimd.memset(dum_a[:], 0.5)
            nc.gpsimd.memset(dum_b[:], 0.25)
            nc.gpsimd.memset(dum_x[:], 0.0)
            nc.gpsimd.memset(magic_sb[:], 0x5f3759df)
            # 4-quadrant warmup windows: full-array utilization is required
            # for the DVFS to ramp the PE clock (1-quadrant work stays at 1/2)
            dpb = [pspool.tile([128, 512], f32, tag="pb", name=f"dm{R}")
                   for R in range(2)]
            for dmy in range(85):
                for R in range(2):
                    for Cg in range(2):
                        nc.tensor.matmul(
                            dpb[R][64 * Cg:64 * Cg + 64, :],
                            dum_x[64 * R:64 * R + 64, 0:64],
                            dum_x[64 * R:64 * R + 64, 0:512],
                            start=True, stop=True,
                            tile_position=(64 * R, 64 * Cg))
            create_xt(0)
            create_xt(1)
            issue_load(0, nsl=8)           # chunk 0 feeds the critical path
            issue_load(1)
            nc.scalar.dma_start(w_sb[:], w_d[:])
            nc.scalar.dma_start(w2_sb[:], w2_d[:])
            nc.scalar.dma_start(bias_sb[:], b_d[:])
            # preload ACT tables during the DMA wait; order so Copy/Square are
            # warm for chunk-0 stats (Identity/Sqrt reload later if evicted)
            nc.scalar.activation(dum_c[:], dum_b[:],
                                 mybir.ActivationFunctionType.Identity,
                                 bias=dum_b[:, 0:1])
            nc.scalar.activation(dum_c[:], dum_b[:],
                                 mybir.ActivationFunctionType.Sqrt)
            nc.scalar.activation(dum_b[:], dum_a[:],
                                 mybir.ActivationFunctionType.Copy)
            nc.scalar.activation(dum_b[:], dum_a[:],
                                 mybir.ActivationFunctionType.Square)

            # chunk-0 stats: 2048-slices {0,2,4} on ACT accum (sum/sq pairs),
            # {1,3,5,6,7} on DVE bn_stats (20 blocks, alpha=0.625)
            s0 = st[0]
            s0["acc"] = spool.tile([128, 6], f32, tag="acc", name="ac0")
            s0["st6"] = spool.tile([128, 32 * 6], f32, tag="st6", name="s60")
            xt0 = s0["xt"]
            nblk = 0
            for k in range(8):
                sl = xt0[:, W + k * 2048: W + (k + 1) * 2048]
                if k in (0, 2, 4):
                    a = k // 2
                    nc.scalar.activation(trash[0][:, 0:2048], sl,
                                         mybir.ActivationFunctionType.Copy,
                                         accum_out=s0["acc"][:, 2 * a:2 * a + 1])
                    nc.scalar.activation(trash[1][:, 0:2048], sl,
                                         mybir.ActivationFunctionType.Square,
                                         accum_out=s0["acc"][:, 2 * a + 1:2 * a + 2])
                else:
                    for j in range(4):
                        nc.vector.bn_stats(
                            s0["st6"][:, nblk * 6:(nblk + 1) * 6],
                            sl[:, j * 512:(j + 1) * 512])
                        nblk += 1
            emit_load(2)
            emit_chain_mix(0, 20 * 512.0 / HW, 3, 20)
            emit_post_chain(0)
            emit_consts(0)
            # floor-hint the chunk-1 stats AFTER chunk-0's chain: without it
            # the list scheduler (whose DMA model is optimistic) interleaves
            # these slice-gated ops into the chunk-0 chain on the DVE stream
            for slot in range(8):
                with wu(T0 + 2 + 2 * slot):
                    emit_stats_bn(1, slot)

            # ---- steady loop ----
            for cc in range(NCH):
                if cc > 0:
                    with wu(T0 + TC * cc - 4):
                        emit_consts(cc)
                om = opool.tile([128, 4 * 4096], bf16, tag="om", name=f"om{cc}")
                for q in range(16):
                    pb = emit_span_mms(cc, q)
                    emit_evac(cc, q, pb, om)
                    if q == 0 and 1 <= cc < NCH - 2:
                        emit_load(cc + 2)
                    if q == 2 and cc + 1 < NCH:
                        # stream order must be: stats(cc+1) [during chunk
                        # cc-1], chain(cc+1), stats(cc+2) -- chain ops must
                        # never sit ahead of chunk-cc evacs in the ACT stream
                        with wu(T0 + TC * (cc + 1) - 6):
                            emit_chain_bn(cc + 1)
                            emit_post_chain(cc + 1)
                    if 3 <= q <= 10 and cc + 2 < NCH:
                        with wu(T0 + TC * (cc + 1) - 4 + (q - 3)):
                            emit_stats_bn(cc + 2, q - 3)
                    if cc < NCH - 1:
                        if q == 7:
                            emit_fixups(cc, om, 0, 8)
                            emit_out(cc, om, 0)
                    else:
                        if q in (3, 7, 11):
                            emit_fixups(cc, om, q - 3, q + 1)
                            emit_out4(cc, om, q // 4)
                        elif q == 13:
                            emit_fixups(cc, om, 12, 14)
                            emit_out8(cc, om, 6)
                if cc < NCH - 1:
                    emit_fixups(cc, om, 8, 16)
                    emit_out(cc, om, 1)
                else:
                    emit_fixups(cc, om, 14, 16)
                    emit_out8(cc, om, 7)
    nc.compile()
    return nc


def _pack_inputs(x, dw, pw, biases):
    """Host-side: fuse pw o dw, scatter into block-diag 64x64 lhsT tap tiles
    plus 3 static const matrices (A_K/A_L/A_R) per chunk."""
    G = 128
    dwr = dw.reshape(B, G, 4, 4, 3, 3)          # [b, g, m, i, kh, kw]
    pwr = pw.reshape(B, G, 4, 4)                # [b, g, j, m]
    eff = np.einsum('bgjm,bgmikl->bgjikl', pwr, dwr)  # [b, g, j, i, kh, kw]
    # 64x64 block-diag tiles: w_host[b, 64R + 4gl + i, (cc*9+tap)*64 + 4gl + j]
    w_host = np.zeros((B, 128, NCH * NTAP * 64), dtype=np.float32)
    wv = w_host.reshape(B, 2, 16, 4, NCH, NTAP, 16, 4)  # [b,R,gl_k,i,cc,tap,gl_m,j]
    er = eff.reshape(B, NCH, 2, 16, 4, 4, NTAP)         # [b, cc, R, gl, j, i, tap]
    for gl in range(16):
        e = er[:, :, :, gl]                     # [b, cc, R, j, i, tap]
        wv[:, :, gl, :, :, :, gl, :] = e.transpose(0, 2, 4, 1, 5, 3)
    w_hostb = w_host.astype(ml_dtypes.bfloat16)

    # const matrices from the bf16-rounded eff (matches device weights)
    effb = w_hostb.astype(np.float32).reshape(B, 2, 16, 4, NCH, NTAP, 16, 4)
    # recover eff[b, cc, R, gl, j, i, tap] from the block diag
    effr = np.zeros((B, NCH, 2, 16, 4, 4, NTAP), np.float32)
    for gl in range(16):
        # effb[:, :, gl, :, :, :, gl, :] dims: [b, R, i, cc, tap, j]
        effr[:, :, :, gl] = effb[:, :, gl, :, :, :, gl, :].transpose(
            0, 3, 1, 5, 2, 4)
    # A_c[b, cc, R, gl, j, i]: K = sum all taps; L = taps with kw=0 (tapi 0,3,6);
    # R = taps with kw=2 (tapi 2,5,8).  tap index = kh*3 + kw.
    A_K = effr.sum(-1)
    A_L = effr[..., [0, 3, 6]].sum(-1)
    A_R = effr[..., [2, 5, 8]].sum(-1)
    # w2[b, 64Rh + 4gl + i, (cc*3+c)*128 + 64h + 4gl + j] = A_c (dup col halves)
    w2 = np.zeros((B, 128, NCH * 3 * 128), dtype=np.float32)
    w2v = w2.reshape(B, 2, 16, 4, NCH, 3, 2, 16, 4)  # [b,Rh,gl,i,cc,c,h,gl_m,j]
    for c, A in enumerate((A_K, A_L, A_R)):
        At = A.transpose(0, 2, 3, 1, 5, 4)  # [b, R, gl, cc, i, j]
        for gl in range(16):
            for h in range(2):
                w2v[:, :, gl, :, :, c, h, gl, :] = At[:, :, gl].transpose(
                    0, 1, 3, 2, 4)

    bias_host = np.zeros((B, 128, 10), dtype=np.float32)
    bfull = biases.reshape(B, C)
    p = np.arange(128)
    for cc in range(NCH):
        for R in range(2):
            bias_host[:, :, cc * 2 + R] = bfull[:, cc * 128 + 64 * R + (p % 64)]
    bias_host[:, 0:64, 8] = 1.0     # R-half masks for the mrr build
    bias_host[:, 64:128, 9] = 1.0
    return w_hostb, w2.astype(ml_dtypes.bfloat16), bias_host


def kernel(x, dw_kernels, pw_kernels, biases):
    from concourse.bass_utils import run_bass_kernel_spmd

    x = np.ascontiguousarray(np.asarray(x, dtype=np.float32))
    dw = np.asarray(dw_kernels, dtype=np.float32)
    pw = np.asarray(pw_kernels, dtype=np.float32)
    bs = np.asarray(biases, dtype=np.float32)

    if "nc" not in _CACHE:
        _CACHE["nc"] = _build_program()
    nc = _CACHE["nc"]

    w_host, w2_host, bias_host = _pack_inputs(x, dw, pw, bs)
    xb = x.reshape(B, C, HW).astype(ml_dtypes.bfloat16)
    in_maps = [{"x": xb[i],
                "w": w_host[i],
                "w2": w2_host[i],
                "bias": bias_host[i]} for i in range(B)]
    res = run_bass_kernel_spmd(nc, in_maps, core_ids=list(range(B)),
                               trace=bool(int(os.environ.get("KTRACE", "0"))))
    _CACHE["last_result"] = res
    out = np.stack([res.results[i]["out"].astype(np.float32).reshape(C, H, W)
                    for i in range(B)])
    return out
